# revision 38
# baseline (speedup 1.0000x reference)
"""Distributed Trainium2 Bass kernel for the quad-masked variance loss
(nn_Cons_Loss_79027398246842), SPMD across 8 NeuronCores.

Math: the quads are axis-aligned rectangles, so the point-in-polygon mask
separates into row_mask[q,h] * col_mask[q,w].  With s1/s2/cnt the masked
sums of pred / pred^2 / 1 per quad, the loss is
    sum_{l,q} where(cnt>0, (s2 - 2*mean*s1 + mean^2*cnt)/max(cnt,1), 0),
    mean = s1/max(cnt,1).

Sharding: W (columns) split across the 8 cores (64 columns each).  Each
core computes partial (s1[l,q], s2[l,q], cnt[q]) over its columns for ALL
64 quads via a two-stage contraction:
  stage 1 (TensorE, bf16): contract H in 4 chunks of 128 rows with the
    transposed row mask as the stationary operand,
  stage 2 (VectorE): multiply by the column mask and reduce over W.
The per-core [64, 9] partials are gathered host-side and the final tiny
reduction (8-way sum + ~30 scalar ops) happens at unshard time — an
on-device AllGather measured ~55us of rank-skew barrier + collective
floor, dwarfing the ~2us of real work in this kernel.

The kernel is raw bass (manual semaphores, no TileContext) to avoid the
Tile init/exit barrier butterflies.  Engine plan per core:
  sync   : aux DMA + 4 per-chunk pred DMAs
  scalar : gt DMA, ACT table warmups, per-chunk (gt>0) via Sign and
           square, out DMA + completion signal
  vector : batched row/col mask comparisons, per-chunk (gt>0)*pred,
           stage-2 colM multiply + W-reduce
  gpsimd : mask AND-combines, end-of-run semaphore cleanup (leaves all
           sems at 0 so the NEFF can be re-executed)
  tensor : per-chunk [s1|s2] (N=512) and cnt (N=64) matmuls, bf16

Semaphore ledger (cumulative):
  sV: t1a=1 t2a=2 c1=3 c2=4 gp0..3=5..8 M12=9 Mg=10 reduce=11
  sQ: rta=1 colM=2
  sS: gC0..3=1..4 sq0..3=5..8
  sT: last-mm=1
  dA/dG/dP0..3/dO: DMA completions (+16 each)
"""
import numpy as np
from contextlib import ExitStack

from concourse import bacc, bass
import concourse.mybir as mybir

F32 = mybir.dt.float32
BF16 = mybir.dt.bfloat16
ALU = mybir.AluOpType

N_CORES = 8
L, H, W = 4, 512, 512
NB = 64
WL = W // N_CORES          # 64 columns per core
HC = 128                   # h-chunk (partition dim)
NCH = H // HC              # 4 chunks
NT = 2 * L + 1             # 9 partial tensors: s1 x4, s2 x4, cnt
EPS = 1e-5

# aux2 input layout [128, 200] f32 (host-prepared constants):
#   [:, 0:64]    lo row broadcast (row-mask lower bound per quad)
#   [:, 64:128]  hi row broadcast
#   [0:64, 128]  x0 - WL*core   [0:64, 129]  x1 - WL*core
#   [:, 130:134] pycol[p, c] = 128*c + p
#   [0:64, 136:200] px grid row: arange(WL) per partition
AUX2_W = 200


def build_kernel(cleanup=True):
    nc = bacc.Bacc("TRN2", target_bir_lowering=False, debug=False,
                   enable_asserts=False)

    pred_e = nc.dram_tensor("pred", [HC, NCH, L, WL], F32, kind="ExternalInput")
    gt_e = nc.dram_tensor("gt", [HC, NCH, WL], F32, kind="ExternalInput")
    aux_e = nc.dram_tensor("aux2", [HC, AUX2_W], F32, kind="ExternalInput")
    out_e = nc.dram_tensor("out", [NB, NT], F32, kind="ExternalOutput")

    ctx = ExitStack()
    sem = lambda name: ctx.enter_context(nc.semaphore(name))
    sb = lambda name, shape, dt=F32: ctx.enter_context(
        nc.sbuf_tensor(name, shape, dt))
    ps = lambda name, shape: ctx.enter_context(
        nc.psum_tensor(name, shape, F32))

    with ctx:
        dA = sem("dA"); dG = sem("dG"); dO = sem("dO")
        dPs = [sem(f"dP{c}") for c in range(NCH)]
        sV = sem("sV"); sS = sem("sS"); sT = sem("sT"); sQ = sem("sQ")
        all_sems = [dA, dG, dO, sV, sS, sT, sQ] + dPs

        AX = sb("AX", [HC, AUX2_W])
        PR = sb("PR", [HC, NCH, L, WL])
        GT = sb("GT", [HC, NCH, WL])
        t1a = sb("t1a", [HC, NCH, NB], BF16)
        t2a = sb("t2a", [HC, NCH, NB], BF16)
        c1 = sb("c1", [NB, WL])
        c2 = sb("c2", [NB, WL])
        colM = sb("colM", [NB, WL])
        rta = sb("rta", [HC, NCH, NB], BF16)
        gpas = [sb(f"gpa{c}", [HC, NT, WL], BF16) for c in range(NCH)]
        M = sb("M", [NB, NT, WL])
        partial = sb("partial", [NB, NT])
        scratch = sb("scratch", [1, 8])

        D12 = ps("D12", [NB, 2 * L, WL])
        Dg = ps("Dg", [NB, WL])

        lo_b = AX[:, 0:NB]
        hi_b = AX[:, NB:2 * NB]
        x0p = AX[0:NB, 128:129]
        x1p = AX[0:NB, 129:130]
        px_b = AX[0:NB, 136:200]

        sv_gp = {c: 5 + c for c in range(NCH)}

        with nc.Block() as block:

            @block.sync
            def _(sync):
                sync.dma_start(out=AX[:, :], in_=aux_e[:, :]).then_inc(dA, 16)
                for c in range(NCH):
                    sync.dma_start(
                        out=PR[:, c, :, :], in_=pred_e[:, c, :, :]
                    ).then_inc(dPs[c], 16)

            @block.vector
            def _(vector):
                def gp(c):
                    gt_bcast = GT[:, c, :].unsqueeze(1).broadcast_to(
                        (HC, L, WL))
                    vector.scalar_tensor_tensor(
                        out=gpas[c][:, 0:L, :], in0=gt_bcast, scalar=0.0,
                        in1=PR[:, c, :, :], op0=ALU.is_gt, op1=ALU.mult,
                    ).then_inc(sV)

                vector.wait_ge(dA, 16)
                lo4 = lo_b.unsqueeze(1).broadcast_to((HC, NCH, NB))
                hi4 = hi_b.unsqueeze(1).broadcast_to((HC, NCH, NB))
                py4 = AX[:, 130:134].unsqueeze(2).broadcast_to((HC, NCH, NB))
                vector.tensor_tensor(
                    out=t1a[:, :, :], in0=lo4, in1=py4, op=ALU.is_le,
                ).then_inc(sV)                                   # sV=1
                vector.tensor_tensor(
                    out=t2a[:, :, :], in0=hi4, in1=py4, op=ALU.is_ge,
                ).then_inc(sV)                                   # sV=2
                vector.tensor_scalar(
                    out=c1[:, :], in0=px_b, scalar1=x0p,
                    scalar2=None, op0=ALU.is_ge,
                ).then_inc(sV)                                   # sV=3
                vector.tensor_scalar(
                    out=c2[:, :], in0=px_b, scalar1=x1p,
                    scalar2=None, op0=ALU.is_le,
                ).then_inc(sV)                                   # sV=4
                vector.wait_ge(dG, 16)
                for c in range(NCH):
                    vector.wait_ge(dPs[c], 16)
                    gp(c)                                        # sV=5+c

                # stage 2: colM multiply + w-reduce
                vector.wait_ge(sT, 1)
                vector.wait_ge(sQ, 2)
                col_bcast = colM[:, :].unsqueeze(1).broadcast_to(
                    (NB, 2 * L, WL))
                vector.tensor_tensor(
                    out=M[:, 0:2 * L, :], in0=D12[:, :, :], in1=col_bcast,
                    op=ALU.mult,
                ).then_inc(sV)                                   # sV=9
                vector.tensor_tensor(
                    out=M[:, 2 * L, :], in0=Dg[:, :], in1=colM[:, :],
                    op=ALU.mult,
                ).then_inc(sV)                                   # sV=10
                # self-sem instead of drain: then_inc fires once the
                # writes have landed, so this orders the M reads below
                vector.wait_ge(sV, 10)
                vector.tensor_reduce(
                    out=partial[:, :], in_=M[:, :, :],
                    axis=mybir.AxisListType.X, op=ALU.add,
                ).then_inc(sV)                                   # sV=11

            @block.gpsimd
            def _(gpsimd):
                gpsimd.wait_ge(sV, 2)
                gpsimd.tensor_tensor(
                    out=rta[:, :, :], in0=t1a[:, :, :], in1=t2a[:, :, :],
                    op=ALU.mult,
                ).then_inc(sQ)                                   # sQ=1
                gpsimd.wait_ge(sV, 4)
                gpsimd.tensor_tensor(
                    out=colM[:, :], in0=c1[:, :], in1=c2[:, :], op=ALU.mult,
                ).then_inc(sQ)                                   # sQ=2
                # hold the kernel open until the out DMA lands; pool is
                # the ONLY dO waiter, so clearing after the wait is safe
                gpsimd.wait_ge(dO, 16)
                if cleanup:
                    gpsimd.dma_reset()
                    lo = min(s.num for s in all_sems)
                    hi = max(s.num for s in all_sems)
                    gpsimd.sem_clear(range(lo, hi + 1))

            @block.scalar
            def _(scalar):
                scalar.dma_start(out=GT[:, :, :], in_=gt_e[:, :, :]).then_inc(
                    dG, 16)
                # pull the ACT square+sign table loads off the critical
                # path; read DMA-initialized SBUF only (uninitialized SBUF
                # reads can take the device down)
                scalar.wait_ge(dG, 16)
                scalar.square(out=scratch[:, 4:5], in_=GT[0:1, 0, 0:1])
                scalar.sign(out=scratch[:, 5:6], in_=GT[0:1, 0, 0:1])
                for c in range(NCH):
                    # gC = sign(gt) == (gt > 0) for non-negative gt
                    scalar.sign(
                        out=gpas[c][:, 2 * L, :], in_=GT[:, c, :],
                    ).then_inc(sS)                               # sS=c+1
                for c in range(NCH):
                    scalar.wait_ge(sV, sv_gp[c])
                    scalar.square(
                        out=gpas[c][:, L:2 * L, :], in_=gpas[c][:, 0:L, :]
                    ).then_inc(sS)                               # sS=5+c
                scalar.wait_ge(sV, 11)
                scalar.dma_start(out=out_e[:, :], in_=partial[:, :]).then_inc(
                    dO, 16)

            @block.tensor
            def _(tensor):
                tensor.wait_ge(sQ, 1)
                for c in range(NCH):
                    tensor.wait_ge(sS, 5 + c)
                    st = dict(start=(c == 0), stop=(c == NCH - 1))
                    tensor.matmul(
                        D12[:, :, :], rta[:, c, :], gpas[c][:, 0:2 * L, :],
                        **st)
                    mm = tensor.matmul(
                        Dg[:, :], rta[:, c, :], gpas[c][:, 2 * L, :], **st)
                    if c == NCH - 1:
                        mm.then_inc(sT)                          # sT=1

    nc.compile()
    return nc


_NC = None


def _get_nc():
    global _NC
    if _NC is None:
        _NC = build_kernel()
    return _NC


def _make_aux(boxes, core):
    aux2 = np.zeros((HC, AUX2_W), dtype=np.float32)
    eps_q = np.float32(2.0 * EPS) / (boxes[:, 2] - boxes[:, 0])
    aux2[:, 0:NB] = boxes[:, 1] + eps_q          # lo row, all partitions
    aux2[:, NB:2 * NB] = boxes[:, 5] - eps_q     # hi row
    aux2[0:NB, 128] = boxes[:, 0] - WL * core    # x0 in core-local coords
    aux2[0:NB, 129] = boxes[:, 2] - WL * core    # x1 in core-local coords
    aux2[:, 130:134] = (
        np.arange(H, dtype=np.float32).reshape(NCH, HC).T)  # pycol
    aux2[0:NB, 136:200] = np.arange(WL, dtype=np.float32)[None, :]
    return aux2


def make_in_maps(pred, gt, boxes):
    pred = np.asarray(pred, dtype=np.float32)
    gt = np.asarray(gt, dtype=np.float32)
    boxes = np.asarray(boxes, dtype=np.float32).reshape(NB, 8)
    # [1,L,H,W] -> per core [HC, NCH, L, WL] (h-within-chunk on partitions)
    pred_c = np.ascontiguousarray(
        pred[0].reshape(L, NCH, HC, W).transpose(2, 1, 0, 3))
    gt_c = np.ascontiguousarray(gt[0].reshape(NCH, HC, W).transpose(1, 0, 2))
    in_maps = []
    for i in range(N_CORES):
        ws = slice(WL * i, WL * (i + 1))
        in_maps.append({
            "pred": np.ascontiguousarray(pred_c[:, :, :, ws]),
            "gt": np.ascontiguousarray(gt_c[:, :, ws]),
            "aux2": _make_aux(boxes, i),
        })
    return in_maps


def finish(partials):
    """Host-side unshard: sum per-core partials and apply the loss formula."""
    tot = np.sum(np.stack(partials, 0), axis=0)  # [NB, 9]
    s1 = tot[:, 0:L].T        # [L, NB]
    s2 = tot[:, L:2 * L].T
    cnt = tot[:, 2 * L]
    safe = np.maximum(cnt, 1.0)
    mean = s1 / safe[None, :]
    per = (s2 - 2.0 * mean * s1 + mean * mean * cnt[None, :]) / safe[None, :]
    per = np.where(cnt[None, :] > 0, per, 0.0)
    return np.float32(per.sum(dtype=np.float32))


def kernel(pred, gt, boxes):
    from concourse.bass_utils import run_bass_kernel_spmd

    nc = _get_nc()
    in_maps = make_in_maps(pred, gt, boxes)
    res = run_bass_kernel_spmd(nc, in_maps, core_ids=list(range(N_CORES)))
    return finish([r["out"] for r in res.results])


if __name__ == "__main__":
    build_kernel()
    print("build + compile OK")


# revision 40
# speedup vs baseline: 1.0164x; 1.0164x over previous
"""Distributed Trainium2 Bass kernel for the quad-masked variance loss
(nn_Cons_Loss_79027398246842), SPMD across 8 NeuronCores.

Math: the quads are axis-aligned rectangles, so the point-in-polygon mask
separates into row_mask[q,h] * col_mask[q,w].  With s1/s2/cnt the masked
sums of pred / pred^2 / 1 per quad, the loss is
    sum_{l,q} where(cnt>0, (s2 - 2*mean*s1 + mean^2*cnt)/max(cnt,1), 0),
    mean = s1/max(cnt,1).

Sharding: W (columns) split across the 8 cores (64 columns each).  Each
core computes partial (s1[l,q], s2[l,q], cnt[q]) over its columns for ALL
64 quads via a two-stage contraction:
  stage 1 (TensorE, bf16): contract H in 4 chunks of 128 rows with the
    transposed row mask as the stationary operand,
  stage 2 (VectorE): multiply by the column mask and reduce over W.
The per-core [64, 9] partials are gathered host-side and the final tiny
reduction (8-way sum + ~30 scalar ops) happens at unshard time — an
on-device AllGather measured ~55us of rank-skew barrier + collective
floor, dwarfing the ~2us of real work in this kernel.

The kernel is raw bass (manual semaphores, no TileContext) to avoid the
Tile init/exit barrier butterflies.  Engine plan per core:
  sync   : aux DMA + 4 per-chunk pred DMAs
  scalar : gt DMA, ACT table warmups, per-chunk (gt>0) via Sign and
           square, out DMA + completion signal
  vector : batched row/col mask comparisons, per-chunk (gt>0)*pred,
           stage-2 colM multiply + W-reduce
  gpsimd : mask AND-combines, end-of-run semaphore cleanup (leaves all
           sems at 0 so the NEFF can be re-executed)
  tensor : per-chunk [s1|s2] (N=512) and cnt (N=64) matmuls, bf16

Semaphore ledger (cumulative):
  sV: t1a=1 t2a=2 c1=3 c2=4 gp0..3=5..8 M12=9 Mg=10 reduce=11
  sQ: rta=1 colM=2
  sS: gC0..3=1..4 sq0..3=5..8
  sT: last-mm=1
  dA/dG/dP0..3/dO: DMA completions (+16 each)
"""
import numpy as np
from contextlib import ExitStack

from concourse import bacc, bass
import concourse.mybir as mybir

F32 = mybir.dt.float32
BF16 = mybir.dt.bfloat16
ALU = mybir.AluOpType

N_CORES = 8
L, H, W = 4, 512, 512
NB = 64
WL = W // N_CORES          # 64 columns per core
HC = 128                   # h-chunk (partition dim)
NCH = H // HC              # 4 chunks
NT = 2 * L + 1             # 9 partial tensors: s1 x4, s2 x4, cnt
EPS = 1e-5

# aux2 input layout [128, 200] f32 (host-prepared constants):
#   [:, 0:64]    lo row broadcast (row-mask lower bound per quad)
#   [:, 64:128]  hi row broadcast
#   [0:64, 128]  x0 - WL*core   [0:64, 129]  x1 - WL*core
#   [:, 130:134] pycol[p, c] = 128*c + p
#   [0:64, 136:200] px grid row: arange(WL) per partition
AUX2_W = 200


def build_kernel(cleanup=True):
    nc = bacc.Bacc("TRN2", target_bir_lowering=False, debug=False,
                   enable_asserts=False)

    pred_e = nc.dram_tensor("pred", [HC, NCH, L, WL], F32, kind="ExternalInput")
    gt_e = nc.dram_tensor("gt", [HC, NCH, WL], F32, kind="ExternalInput")
    aux_e = nc.dram_tensor("aux2", [HC, AUX2_W], F32, kind="ExternalInput")
    out_e = nc.dram_tensor("out", [NB, NT], F32, kind="ExternalOutput")

    ctx = ExitStack()
    sem = lambda name: ctx.enter_context(nc.semaphore(name))
    sb = lambda name, shape, dt=F32: ctx.enter_context(
        nc.sbuf_tensor(name, shape, dt))
    ps = lambda name, shape: ctx.enter_context(
        nc.psum_tensor(name, shape, F32))

    with ctx:
        dA = sem("dA"); dG = sem("dG"); dO = sem("dO")
        dPs = [sem(f"dP{c}") for c in range(NCH)]
        sV = sem("sV"); sS = sem("sS"); sT = sem("sT"); sQ = sem("sQ")
        all_sems = [dA, dG, dO, sV, sS, sT, sQ] + dPs

        AX = sb("AX", [HC, AUX2_W])
        PR = sb("PR", [HC, NCH, L, WL])
        GT = sb("GT", [HC, NCH, WL])
        t1a = sb("t1a", [HC, NCH, NB], BF16)
        t2a = sb("t2a", [HC, NCH, NB], BF16)
        c1 = sb("c1", [NB, WL])
        c2 = sb("c2", [NB, WL])
        colM = sb("colM", [NB, WL])
        rta = sb("rta", [HC, NCH, NB], BF16)
        gpas = [sb(f"gpa{c}", [HC, NT, WL], BF16) for c in range(NCH)]
        M = sb("M", [NB, NT, WL])
        partial = sb("partial", [NB, NT])
        scratch = sb("scratch", [1, 8])

        D12 = ps("D12", [NB, 2 * L, WL])
        Dg = ps("Dg", [NB, WL])

        lo_b = AX[:, 0:NB]
        hi_b = AX[:, NB:2 * NB]
        x0p = AX[0:NB, 128:129]
        x1p = AX[0:NB, 129:130]
        px_b = AX[0:NB, 136:200]

        sv_gp = {c: 5 + c for c in range(NCH)}

        with nc.Block() as block:

            @block.sync
            def _(sync):
                sync.dma_start(out=AX[:, :], in_=aux_e[:, :]).then_inc(dA, 16)
                for c in range(NCH):
                    sync.dma_start(
                        out=PR[:, c, :, :], in_=pred_e[:, c, :, :]
                    ).then_inc(dPs[c], 16)

            @block.vector
            def _(vector):
                def gp(c):
                    gt_bcast = GT[:, c, :].unsqueeze(1).broadcast_to(
                        (HC, L, WL))
                    vector.scalar_tensor_tensor(
                        out=gpas[c][:, 0:L, :], in0=gt_bcast, scalar=0.0,
                        in1=PR[:, c, :, :], op0=ALU.is_gt, op1=ALU.mult,
                    ).then_inc(sV)

                vector.wait_ge(dA, 16)
                lo4 = lo_b.unsqueeze(1).broadcast_to((HC, NCH, NB))
                hi4 = hi_b.unsqueeze(1).broadcast_to((HC, NCH, NB))
                py4 = AX[:, 130:134].unsqueeze(2).broadcast_to((HC, NCH, NB))
                vector.tensor_tensor(
                    out=t1a[:, :, :], in0=lo4, in1=py4, op=ALU.is_le,
                ).then_inc(sV)                                   # sV=1
                vector.tensor_tensor(
                    out=t2a[:, :, :], in0=hi4, in1=py4, op=ALU.is_ge,
                ).then_inc(sV)                                   # sV=2
                vector.tensor_scalar(
                    out=c1[:, :], in0=px_b, scalar1=x0p,
                    scalar2=None, op0=ALU.is_ge,
                ).then_inc(sV)                                   # sV=3
                vector.tensor_scalar(
                    out=c2[:, :], in0=px_b, scalar1=x1p,
                    scalar2=None, op0=ALU.is_le,
                ).then_inc(sV)                                   # sV=4
                vector.wait_ge(dG, 16)
                for c in range(NCH):
                    vector.wait_ge(dPs[c], 16)
                    gp(c)                                        # sV=5+c

                # stage 2: colM multiply + w-reduce
                vector.wait_ge(sT, 1)
                vector.wait_ge(sQ, 2)
                col_bcast = colM[:, :].unsqueeze(1).broadcast_to(
                    (NB, 2 * L, WL))
                vector.tensor_tensor(
                    out=M[:, 0:2 * L, :], in0=D12[:, :, :], in1=col_bcast,
                    op=ALU.mult,
                ).then_inc(sV)                                   # sV=9
                vector.tensor_tensor(
                    out=M[:, 2 * L, :], in0=Dg[:, :], in1=colM[:, :],
                    op=ALU.mult,
                ).then_inc(sV)                                   # sV=10
                # self-sem instead of drain: then_inc fires once the
                # writes have landed, so this orders the M reads below
                vector.wait_ge(sV, 10)
                vector.tensor_reduce(
                    out=partial[:, :], in_=M[:, :, :],
                    axis=mybir.AxisListType.X, op=ALU.add,
                ).then_inc(sV)                                   # sV=11

            @block.gpsimd
            def _(gpsimd):
                gpsimd.wait_ge(sV, 2)
                gpsimd.tensor_tensor(
                    out=rta[:, :, :], in0=t1a[:, :, :], in1=t2a[:, :, :],
                    op=ALU.mult,
                ).then_inc(sQ)                                   # sQ=1
                gpsimd.wait_ge(sV, 4)
                gpsimd.tensor_tensor(
                    out=colM[:, :], in0=c1[:, :], in1=c2[:, :], op=ALU.mult,
                ).then_inc(sQ)                                   # sQ=2
                # hold the kernel open until the out DMA lands; pool is
                # the ONLY dO waiter, so clearing after the wait is safe
                gpsimd.wait_ge(dO, 16)
                if cleanup:
                    gpsimd.dma_reset()
                    lo = min(s.num for s in all_sems)
                    hi = max(s.num for s in all_sems)
                    gpsimd.sem_clear(range(lo, hi + 1))

            @block.scalar
            def _(scalar):
                scalar.dma_start(out=GT[:, :, :], in_=gt_e[:, :, :]).then_inc(
                    dG, 16)
                # pull the ACT square+sign table loads off the critical
                # path; read DMA-initialized SBUF only (uninitialized SBUF
                # reads can take the device down)
                scalar.wait_ge(dG, 16)
                scalar.square(out=scratch[:, 4:5], in_=GT[0:1, 0, 0:1])
                scalar.sign(out=scratch[:, 5:6], in_=GT[0:1, 0, 0:1])
                for c in range(NCH):
                    # gC = sign(gt) == (gt > 0) for non-negative gt
                    scalar.sign(
                        out=gpas[c][:, 2 * L, :], in_=GT[:, c, :],
                    ).then_inc(sS)                               # sS=c+1
                for c in range(NCH):
                    scalar.wait_ge(sV, sv_gp[c])
                    scalar.square(
                        out=gpas[c][:, L:2 * L, :], in_=gpas[c][:, 0:L, :]
                    ).then_inc(sS)                               # sS=5+c
                scalar.wait_ge(sV, 11)
                scalar.dma_start(out=out_e[:, :], in_=partial[:, :]).then_inc(
                    dO, 16)

            @block.tensor
            def _(tensor):
                tensor.wait_ge(sQ, 1)
                for c in range(NCH):
                    tensor.wait_ge(sS, 5 + c)
                    st = dict(start=(c == 0), stop=(c == NCH - 1))
                    tensor.matmul(
                        D12[:, :, :], rta[:, c, :], gpas[c][:, 0:2 * L, :],
                        **st)
                    mm = tensor.matmul(
                        Dg[:, :], rta[:, c, :], gpas[c][:, 2 * L, :], **st)
                    if c == NCH - 1:
                        mm.then_inc(sT)                          # sT=1

    nc.compile()
    return nc


_NC = None


def _get_nc():
    global _NC
    if _NC is None:
        _NC = build_kernel()
    return _NC


def _make_aux(boxes, core):
    aux2 = np.zeros((HC, AUX2_W), dtype=np.float32)
    eps_q = np.float32(2.0 * EPS) / (boxes[:, 2] - boxes[:, 0])
    aux2[:, 0:NB] = boxes[:, 1] + eps_q          # lo row, all partitions
    aux2[:, NB:2 * NB] = boxes[:, 5] - eps_q     # hi row
    aux2[0:NB, 128] = boxes[:, 0] - WL * core    # x0 in core-local coords
    aux2[0:NB, 129] = boxes[:, 2] - WL * core    # x1 in core-local coords
    aux2[:, 130:134] = (
        np.arange(H, dtype=np.float32).reshape(NCH, HC).T)  # pycol
    aux2[0:NB, 136:200] = np.arange(WL, dtype=np.float32)[None, :]
    return aux2


def make_in_maps(pred, gt, boxes):
    pred = np.asarray(pred, dtype=np.float32)
    gt = np.asarray(gt, dtype=np.float32)
    boxes = np.asarray(boxes, dtype=np.float32).reshape(NB, 8)
    # [1,L,H,W] -> per core [HC, NCH, L, WL] (h-within-chunk on partitions)
    pred_c = np.ascontiguousarray(
        pred[0].reshape(L, NCH, HC, W).transpose(2, 1, 0, 3))
    gt_c = np.ascontiguousarray(gt[0].reshape(NCH, HC, W).transpose(1, 0, 2))
    in_maps = []
    for i in range(N_CORES):
        ws = slice(WL * i, WL * (i + 1))
        in_maps.append({
            "pred": np.ascontiguousarray(pred_c[:, :, :, ws]),
            "gt": np.ascontiguousarray(gt_c[:, :, ws]),
            "aux2": _make_aux(boxes, i),
        })
    return in_maps


def finish(partials):
    """Host-side unshard: sum per-core partials and apply the loss formula."""
    tot = np.sum(np.stack(partials, 0), axis=0)  # [NB, 9]
    s1 = tot[:, 0:L].T        # [L, NB]
    s2 = tot[:, L:2 * L].T
    cnt = tot[:, 2 * L]
    safe = np.maximum(cnt, 1.0)
    mean = s1 / safe[None, :]
    per = (s2 - 2.0 * mean * s1 + mean * mean * cnt[None, :]) / safe[None, :]
    per = np.where(cnt[None, :] > 0, per, 0.0)
    return np.float32(per.sum(dtype=np.float32))


def kernel(pred, gt, boxes):
    from concourse.bass_utils import run_bass_kernel_spmd

    nc = _get_nc()
    in_maps = make_in_maps(pred, gt, boxes)
    res = run_bass_kernel_spmd(nc, in_maps, core_ids=list(range(N_CORES)))
    return finish([r["out"] for r in res.results])


if __name__ == "__main__":
    build_kernel()
    print("build + compile OK")
